# revision 1
# baseline (speedup 1.0000x reference)
"""Trainium2 Bass kernel for nn_BlockSparseMLP (MoE gated MLP, E=8, top-2).

Strategy: expert parallelism over 8 NeuronCores. The router matmul
(x @ w_router, 67 MFLOP out of the 206 GFLOP total) plus the top-2
dispatch/gather and the final scatter-add combine run on the host; each
core runs the full gated MLP (gate/up, silu*up, down, weighted by the
routing prob) for the tokens routed to its expert.

All matmul operands are bf16 (fp32 PSUM accumulation): same PE rate as
float32r on trn2 but half the HBM traffic (~55 MB/core vs ~109), which
takes the DMA subsystem off the critical path. End-to-end rel err vs
the fp32 reference is ~4e-3 (tolerance 2e-2).

Per-core device layout (capacity B0=512 tokens, token-major tiles of
128; k-tiles are DMA'd in pairs so every transfer is a contiguous
[128 x 2KB-per-partition] run):
  phase 1 (gate/up): stream w_gate/w_up in [128, 1024] bf16 tiles
    (two k-tiles, each 256 gate | 256 up); weights stationary, xT
    moving (N=512); silu(gate)*up fused on ACT+DVE into aT ([I, tok]
    bf16 layout, resident in SBUF).
  phase 2 (down): stream w_down in [128, 1024] bf16 tiles (two
    k-tiles), aT tiles stationary, accumulate over I into [tok, 512]
    psum tiles, scale by the per-token routing weight, DMA out fp32.
"""

import sys
import functools

sys.path.insert(0, "/opt/trn_rl_repo")

import numpy as np
import ml_dtypes

BF16 = ml_dtypes.bfloat16

T, H, II, E, TOPK = 2048, 2048, 4096, 8, 2
NCORES = 8
B0 = 512        # per-expert token capacity (moving N)
CHUNK = 256     # phase-1 weight chunk width along I
KT = H // 128   # 16 contraction tiles for gate/up
MTI = II // 128  # 32 I tiles
NMC = II // CHUNK  # 16 weight chunks
JJ = CHUNK // 128  # 2 m-tiles per chunk
KI = II // 128  # 32 contraction tiles for down
NH = H // 512   # 4 output column chunks
NT = B0 // 128  # 4 token tiles


@functools.lru_cache(maxsize=2)
def _build(nb1: int):
    """Build the SPMD Bass program (bf16 operands, fp32 accumulate)."""
    assert nb1 == 0
    import concourse.mybir as mybir
    import concourse.tile as tile
    from concourse import bacc

    f32 = mybir.dt.float32
    bf16 = mybir.dt.bfloat16

    nc = bacc.Bacc(None)
    xT = nc.declare_dram_parameter("xT0", [KT // 2, 128, 2 * B0], bf16, isOutput=False)
    wgu = nc.declare_dram_parameter("wgu", [NMC, KT // 2, 128, 4 * CHUNK], bf16, isOutput=False)
    wd = nc.declare_dram_parameter("wd", [NH, KI // 2, 128, 1024], bf16, isOutput=False)
    rw = nc.declare_dram_parameter("rw", [128, NT], f32, isOutput=False)
    dout = nc.declare_dram_parameter("d", [NT, 128, H], f32, isOutput=True)

    SILU = mybir.ActivationFunctionType.Silu

    with tile.TileContext(nc) as tc:
        with (
            tc.tile_pool(name="pers", bufs=1) as pers,
            tc.tile_pool(name="wpool", bufs=32) as wpool,
        ):
            aT0 = pers.tile([128, MTI, B0], bf16)
            rwt = pers.tile([128, NT], f32)
            nc.gpsimd.dma_start(rwt[:], rw[:])

            with (
                tc.tile_pool(name="xp", bufs=1) as xp,
                tc.tile_pool(name="ps1", bufs=1, space="PSUM") as ps1,
                tc.tile_pool(name="sp", bufs=2) as sp,
            ):
                xt = [xp.tile([128, 2 * B0], bf16, name=f"xt_{i}")
                      for i in range(KT // 2)]

                # Head supply: xt and mc0's weights alternate on sync/scalar
                # in consumption order; mc1/mc2's weights go out in parallel
                # on the otherwise-idle gpsimd queue (per-tile transfers so
                # the runtime spreads them across its DMA queue pool — one
                # big transfer would pin to a single ~75 GB/s queue).
                wb_pre = {}
                for i in range(KT // 2):
                    eng_x = nc.sync if i % 2 == 0 else nc.scalar
                    eng_w = nc.scalar if i % 2 == 0 else nc.sync
                    eng_x.dma_start(xt[i][:], xT[i])
                    wbk = wpool.tile([128, 4 * CHUNK], bf16, name="wbk", tag="w")
                    eng_w.dma_start(wbk[:], wgu[0, i])
                    wb_pre[(0, i)] = wbk
                for mcp in (1, 2):
                    for i in range(KT // 2):
                        wbk = wpool.tile([128, 4 * CHUNK], bf16,
                                         name="wbk", tag="w")
                        nc.gpsimd.dma_start(wbk[:], wgu[mcp, i])
                        wb_pre[(mcp, i)] = wbk

                for mc in range(NMC):
                    if mc < 3:
                        wb = [wb_pre[(mc, i)] for i in range(KT // 2)]
                    else:
                        wb = []
                        for i in range(KT // 2):
                            eng_w = nc.scalar if i % 2 == 0 else nc.sync
                            wbk = wpool.tile([128, 4 * CHUNK], bf16,
                                             name="wbk", tag="w")
                            eng_w.dma_start(wbk[:], wgu[mc, i])
                            wb.append(wbk)

                    pg0 = [ps1.tile([128, B0], f32, name="pg0", tag="pg0", bufs=3)
                           for _ in range(JJ)]
                    pu0 = [ps1.tile([128, B0], f32, name="pu0", tag="pu0", bufs=3)
                           for _ in range(JJ)]

                    # The j0/j1 stationary tiles are column-interleaved in
                    # SBUF (host packs them that way): a stride-2 weight AP
                    # keeps walrus off the 4-XBUS FWL load path, which
                    # otherwise steals stream bandwidth from the moving
                    # operand (259 -> ~227 ns per matmul).
                    for k in range(KT):
                        i, half = divmod(k, 2)
                        off = half * 2 * CHUNK
                        st = dict(start=(k == 0), stop=(k == KT - 1))
                        rhs = xt[i][:, half * B0:(half + 1) * B0]
                        for j in range(JJ):
                            nc.tensor.matmul(
                                pg0[j][:],
                                wb[i][:, off + j:off + j + CHUNK - 1:2],
                                rhs, **st
                            )
                        for j in range(JJ):
                            nc.tensor.matmul(
                                pu0[j][:],
                                wb[i][:, off + CHUNK + j:off + CHUNK + j + CHUNK - 1:2],
                                rhs, **st
                            )

                    for j in range(JJ):
                        m = mc * JJ + j
                        sg = sp.tile([128, B0], f32, name="sg", tag="sg")
                        nc.scalar.activation(sg[:], pg0[j][:], SILU)
                        # aT is written token-interleaved (quarter q of the
                        # 512 tokens lands at columns (q//2)*256 + (q%2) :: 2)
                        # so phase 2's stationary reads are also stride-2.
                        for q in range(4):
                            dst = aT0[:, m, (q // 2) * 256 + (q % 2):
                                      (q // 2) * 256 + (q % 2) + 255:2]
                            nc.vector.tensor_mul(
                                dst,
                                sg[:, q * 128:(q + 1) * 128],
                                pu0[j][:, q * 128:(q + 1) * 128],
                            )

            with (
                tc.tile_pool(name="ps2", bufs=1, space="PSUM") as ps2,
                tc.tile_pool(name="op", bufs=3) as op,
            ):
                    for nh in range(NH):
                        pd = [ps2.tile([128, 512], f32, name="pd", tag="pd", bufs=7)
                              for _ in range(NT)]
                        wdk_cur = None
                        for k in range(KI):
                            i, half = divmod(k, 2)
                            if half == 0:
                                wdk_cur = wpool.tile([128, 1024], bf16,
                                                     name="wdk", tag="w")
                                eng = nc.sync if i % 2 == 0 else nc.scalar
                                eng.dma_start(wdk_cur[:], wd[nh, i])
                            st = dict(start=(k == 0), stop=(k == KI - 1))
                            mv = wdk_cur[:, half * 512:(half + 1) * 512]
                            for mt in range(NT):
                                base = (mt // 2) * 256 + (mt % 2)
                                nc.tensor.matmul(
                                    pd[mt][:], aT0[:, k, base:base + 255:2], mv, **st
                                )
                        for mt in range(NT):
                            ot = op.tile([128, 512], f32, name="ot", tag="ot")
                            nc.vector.tensor_scalar_mul(ot[:], pd[mt][:],
                                                        rwt[:, mt:mt + 1])
                            nc.gpsimd.dma_start(
                                dout[mt][:, nh * 512:(nh + 1) * 512], ot[:]
                            )

    nc.compile()
    return nc


@functools.lru_cache(maxsize=2)
def _get_exec(nb1: int):
    """Compile the Bass program and return (nc, run_fn) with a cached jit.

    run_fn(in_maps) -> list of per-core {"d": np.ndarray}. Mirrors
    bass2jax.run_bass_via_pjrt's multi-core branch, but keeps the jitted
    function alive across kernel() calls so repeat invocations skip XLA
    and NEFF compilation.
    """
    import jax
    import concourse.mybir as mybir
    from concourse import bass2jax

    nc = _build(nb1)
    bass2jax.install_neuronx_cc_hook()

    partition_name = nc.partition_id_tensor.name if nc.partition_id_tensor else None
    in_names, out_names, out_avals = [], [], []
    zero_out_shapes = []
    for alloc in nc.m.functions[0].allocations:
        if not isinstance(alloc, mybir.MemoryLocationSet):
            continue
        name = alloc.memorylocations[0].name
        if alloc.kind == "ExternalInput":
            if name != partition_name:
                in_names.append(name)
        elif alloc.kind == "ExternalOutput":
            shape = tuple(alloc.tensor_shape)
            dtype = mybir.dt.np(alloc.dtype)
            out_names.append(name)
            out_avals.append(jax.core.ShapedArray(shape, dtype))
            zero_out_shapes.append((shape, dtype))
    n_params = len(in_names)
    n_outs = len(out_names)
    all_names = list(in_names) + list(out_names)
    if partition_name is not None:
        all_names.append(partition_name)
    donate = tuple(range(n_params, n_params + n_outs))

    def _body(*args):
        operands = list(args)
        if partition_name is not None:
            operands.append(bass2jax.partition_id_tensor())
        outs = bass2jax._bass_exec_p.bind(
            *operands,
            out_avals=tuple(out_avals),
            in_names=tuple(all_names),
            out_names=tuple(out_names),
            lowering_input_output_aliases=(),
            sim_require_finite=True,
            sim_require_nnan=True,
            nc=nc,
        )
        return tuple(outs)

    devices = jax.devices()[:NCORES]
    assert len(devices) == NCORES, f"need {NCORES} devices, have {len(jax.devices())}"
    mesh = bass2jax.Mesh(np.asarray(devices), ("core",))
    in_specs = (bass2jax.PartitionSpec("core"),) * (n_params + n_outs)
    out_specs = (bass2jax.PartitionSpec("core"),) * n_outs
    sharded = jax.jit(
        bass2jax.shard_map(
            _body, mesh=mesh, in_specs=in_specs, out_specs=out_specs, check_rep=False
        ),
        donate_argnums=donate,
        keep_unused=True,
    )

    def run_fn(in_maps):
        concat_in = [
            np.concatenate([np.asarray(m[name]) for m in in_maps], axis=0)
            for name in in_names
        ]
        zeros = [
            np.zeros((shape[0] * NCORES,) + shape[1:], dtype)
            for shape, dtype in zero_out_shapes
        ]
        out_arrs = sharded(*concat_in, *zeros)
        results = []
        for c in range(NCORES):
            res = {}
            for i, name in enumerate(out_names):
                arr = np.asarray(out_arrs[i])
                per = arr.shape[0] // NCORES
                res[name] = arr[c * per:(c + 1) * per]
            results.append(res)
        return results

    return nc, run_fn


def _route(x, w_router):
    """Top-2 routing: expert ids + softmax weights, matching jax.lax.top_k
    (descending, ties to the lower index) + jax.nn.softmax."""
    logits = x.astype(np.float64) @ w_router.astype(np.float64)
    top2 = np.argsort(-logits, axis=1, kind="stable")[:, :TOPK]
    vals = np.take_along_axis(logits, top2, 1).astype(np.float32)
    e = np.exp(vals - vals.max(axis=1, keepdims=True))
    w = (e / e.sum(axis=1, keepdims=True)).astype(np.float32)
    return top2, w


def _reference_numpy(x, w_router, w_gate, w_up, w_down):
    """Correct-but-slow dense fallback for shapes the device program doesn't cover."""
    x = x.astype(np.float32)
    logits = x @ w_router.astype(np.float32)
    n_exp = w_gate.shape[0]
    k = min(TOPK, n_exp)
    top = np.argsort(-logits, axis=1, kind="stable")[:, :k]
    vals = np.take_along_axis(logits, top, 1)
    ex = np.exp(vals - vals.max(1, keepdims=True))
    ww = (ex / ex.sum(1, keepdims=True)).astype(np.float32)
    w_dense = np.zeros_like(logits)
    t_ids = np.arange(x.shape[0])[:, None]
    w_dense[t_ids, top] = ww
    out = np.zeros((x.shape[0], w_down.shape[-1]), np.float32)
    for e in range(n_exp):
        g = x @ w_gate[e]
        u = x @ w_up[e]
        a = (g / (1.0 + np.exp(-g))) * u
        out += w_dense[:, e:e + 1] * (a @ w_down[e])
    return out


def _pack_core_inputs(x, wg_e, wu_e, wd_e, toks, ws, nb1):
    """Build one core's input map: gathered/transposed tokens + packed bf16
    weight tiles matching the SBUF layout, so each DMA tile is one
    contiguous [128 x 2KB-per-partition] run (k-tiles packed in pairs)."""
    assert nb1 == 0
    n_e = len(toks)
    xg = np.zeros((B0, H), np.float32)
    xg[:n_e] = x[toks]
    xTb = np.ascontiguousarray(xg.T).astype(BF16).reshape(KT, 128, B0)
    xTp = np.ascontiguousarray(
        xTb.reshape(KT // 2, 2, 128, B0).transpose(0, 2, 1, 3)
    ).reshape(KT // 2, 128, 2 * B0)
    rfull = np.zeros(B0, np.float32)
    rfull[:n_e] = ws

    def _ileave(block):
        # [..., 256] laid out j0(128)|j1(128) -> column-interleaved
        # j0[c] at 2c, j1[c] at 2c+1 (stride-2 stationary APs defeat FWL)
        lead = block.shape[:-1]
        return np.ascontiguousarray(
            block.reshape(*lead, 2, 128).swapaxes(-2, -1)
        ).reshape(*lead, 2 * 128)

    wgb = wg_e.astype(BF16).reshape(KT, 128, NMC, CHUNK)
    wub = wu_e.astype(BF16).reshape(KT, 128, NMC, CHUNK)
    wgu = np.empty((NMC, KT, 128, 2 * CHUNK), BF16)
    wgu[..., :CHUNK] = _ileave(wgb.transpose(2, 0, 1, 3))
    wgu[..., CHUNK:] = _ileave(wub.transpose(2, 0, 1, 3))
    wgup = np.ascontiguousarray(
        wgu.reshape(NMC, KT // 2, 2, 128, 2 * CHUNK).transpose(0, 1, 3, 2, 4)
    ).reshape(NMC, KT // 2, 128, 4 * CHUNK)

    wdb = wd_e.astype(BF16).reshape(KI, 128, NH, 512).transpose(2, 0, 1, 3)
    wdp = np.ascontiguousarray(
        wdb.reshape(NH, KI // 2, 2, 128, 512).transpose(0, 1, 3, 2, 4)
    ).reshape(NH, KI // 2, 128, 1024)

    return {
        "xT0": xTp,
        "wgu": wgup,
        "wd": wdp,
        "rw": np.ascontiguousarray(rfull.reshape(NT, 128).T),
    }


def kernel(x, w_router, w_gate, w_up, w_down):
    x = np.ascontiguousarray(np.asarray(x, dtype=np.float32))
    w_router = np.asarray(w_router, dtype=np.float32)
    w_gate = np.ascontiguousarray(np.asarray(w_gate, dtype=np.float32))
    w_up = np.ascontiguousarray(np.asarray(w_up, dtype=np.float32))
    w_down = np.ascontiguousarray(np.asarray(w_down, dtype=np.float32))

    if (x.shape != (T, H) or w_router.shape != (H, E)
            or w_gate.shape != (E, H, II) or w_up.shape != (E, H, II)
            or w_down.shape != (E, II, H)):
        return _reference_numpy(x, w_router, w_gate, w_up, w_down)

    top2, w = _route(x, w_router)
    tok = np.repeat(np.arange(T), TOPK)
    te = top2.ravel()
    tw = w.ravel()
    toks_e, ws_e = [], []
    for e in range(E):
        sel = te == e
        toks_e.append(tok[sel])
        ws_e.append(tw[sel].astype(np.float32))

    # Capacity-factor dispatch: the device program handles up to B0=512
    # tokens per expert (98.5% of routed tokens for balanced routing); the
    # rare spill beyond capacity goes through an exact fp32 host path.
    nc, run_fn = _get_exec(0)

    in_maps = [
        _pack_core_inputs(x, w_gate[e], w_up[e], w_down[e],
                          toks_e[e][:B0], ws_e[e][:B0], 0)
        for e in range(E)
    ]

    try:
        results = run_fn(in_maps)
    except Exception:
        import time as _time
        _time.sleep(20)
        results = run_fn(in_maps)

    out = np.zeros((T, H), np.float32)
    for e in range(E):
        n_e = min(len(toks_e[e]), B0)
        d = results[e]["d"].reshape(B0, H)
        out[toks_e[e][:B0]] += d[:n_e]
        spill = toks_e[e][B0:]
        if spill.size:
            xe = x[spill]
            g = xe @ w_gate[e]
            u = xe @ w_up[e]
            a = (g / (1.0 + np.exp(-g))) * u
            out[spill] += (a @ w_down[e]) * ws_e[e][B0:, None]
    return out



# revision 2
# speedup vs baseline: 1.0119x; 1.0119x over previous
"""Trainium2 Bass kernel for nn_BlockSparseMLP (MoE gated MLP, E=8, top-2).

Strategy: expert parallelism over 8 NeuronCores. The router matmul
(x @ w_router, 67 MFLOP out of the 206 GFLOP total) plus the top-2
dispatch/gather and the final scatter-add combine run on the host; each
core runs the full gated MLP (gate/up, silu*up, down, weighted by the
routing prob) for the tokens routed to its expert.

All matmul operands are bf16 (fp32 PSUM accumulation): same PE rate as
float32r on trn2 but half the HBM traffic (~55 MB/core vs ~109), which
takes the DMA subsystem off the critical path. End-to-end rel err vs
the fp32 reference is ~4e-3 (tolerance 2e-2).

Per-core device layout (capacity B0=512 tokens, token-major tiles of
128; k-tiles are DMA'd in pairs so every transfer is a contiguous
[128 x 2KB-per-partition] run):
  phase 1 (gate/up): 32 chunks of 128 I-columns each; per chunk one
    gate psum + one up psum accumulate over the 16 H k-tiles (2 psum
    chains -> only 4 PSUM banks with a 2-deep rotation), then
    silu(gate)*up fused on ACT+DVE into aT ([I, tok] bf16, resident).
  phase 2 (down): per 512-wide output column block, two token-pair
    halves of 2 psum chains each accumulate over I; phase 2 owns its
    own 4 PSUM banks statically so its first matmuls pipeline straight
    behind phase 1's last one with no bank WAR stall. aT is split into
    4 subtiles so the cross-phase dependency resolves early.
  drains alternate DVE / ACT (ACT applies the routing weight as a
    per-partition Copy scale); output DMA triggers spread over
    gpsimd, with the final two on sync/scalar so the tail chain is
    short. Head DMA triggers fan out over sync/scalar/gpsimd so the
    first matmul starts as soon as the first x/weight tiles land.
"""

import sys
import functools

sys.path.insert(0, "/opt/trn_rl_repo")

import numpy as np
import ml_dtypes

BF16 = ml_dtypes.bfloat16

T, H, II, E, TOPK = 2048, 2048, 4096, 8, 2
NCORES = 8
B0 = 512        # per-expert token capacity (moving N)
CHUNK = 256     # weight chunk width along I (DMA tile granularity)
KT = H // 128   # 16 contraction tiles for gate/up
MTI = II // 128  # 32 I tiles
NMC = II // CHUNK  # 16 weight chunks
JJ = CHUNK // 128  # 2 m-tiles per weight chunk
KI = II // 128  # 32 contraction tiles for down
NH = H // 512   # 4 output column chunks
NT = B0 // 128  # 4 token tiles


@functools.lru_cache(maxsize=2)
def _build(nb1: int):
    """Build the SPMD Bass program (bf16 operands, fp32 accumulate)."""
    assert nb1 == 0
    import concourse.mybir as mybir
    import concourse.tile as tile
    from concourse import bacc

    f32 = mybir.dt.float32
    bf16 = mybir.dt.bfloat16

    nc = bacc.Bacc(None)
    xT = nc.declare_dram_parameter("xT0", [KT // 2, 128, 2 * B0], bf16, isOutput=False)
    wgu = nc.declare_dram_parameter("wgu", [NMC, KT // 2, 128, 4 * CHUNK], bf16, isOutput=False)
    wd = nc.declare_dram_parameter("wd", [NH, KI // 2, 128, 1024], bf16, isOutput=False)
    rw = nc.declare_dram_parameter("rw", [128, NT], f32, isOutput=False)
    dout = nc.declare_dram_parameter("d", [NT, 128, H], f32, isOutput=True)

    SILU = mybir.ActivationFunctionType.Silu

    with tile.TileContext(nc) as tc:
        with (
            tc.tile_pool(name="pers", bufs=1) as pers,
            tc.tile_pool(name="wpool", bufs=32) as wpool,
            tc.tile_pool(name="xp", bufs=1) as xp,
            tc.tile_pool(name="sp", bufs=2) as sp,
            tc.tile_pool(name="op", bufs=6) as op,
            tc.tile_pool(name="ps1", bufs=1, space="PSUM") as ps1,
            tc.tile_pool(name="ps2", bufs=1, space="PSUM") as ps2,
        ):
            # aT split into 4 subtiles so phase 2's early k-tiles don't
            # falsely depend on phase 1's last writes.
            aT = [pers.tile([128, MTI // 4, B0], bf16, name=f"aT_{g}")
                  for g in range(4)]
            rwt = pers.tile([128, NT], f32)

            xt = [xp.tile([128, 2 * B0], bf16, name=f"xt_{i}")
                  for i in range(KT // 2)]

            # Head supply: the first chunk's x tiles and weights fan out
            # over sync/scalar/gpsimd so the k=0 operands land ~2 DMA
            # latencies after sequencer start. Per-tile transfers so the
            # runtime spreads them across its DMA queue pool.
            wb_pre = {}
            wbk = wpool.tile([128, 4 * CHUNK], bf16, name="wbk", tag="w")
            nc.gpsimd.dma_start(wbk[:], wgu[0, 0])
            wb_pre[(0, 0)] = wbk
            nc.gpsimd.dma_start(rwt[:], rw[:])
            for p in range(KT // 2):
                eng = nc.sync if p % 2 == 0 else nc.scalar
                eng.dma_start(xt[p][:], xT[p])
                q = p + 1
                if q < KT // 2:
                    wbk = wpool.tile([128, 4 * CHUNK], bf16, name="wbk", tag="w")
                    eng.dma_start(wbk[:], wgu[0, q])
                    wb_pre[(0, q)] = wbk
            for mcw in (1, 2):
                for i in range(KT // 2):
                    wbk = wpool.tile([128, 4 * CHUNK], bf16, name="wbk", tag="w")
                    nc.gpsimd.dma_start(wbk[:], wgu[mcw, i])
                    wb_pre[(mcw, i)] = wbk

            # ---- phase 1: gate/up + silu*up, 32 chunks of 128 I-cols ----
            # The j0/j1 stationary tiles are column-interleaved in SBUF
            # (host packs them that way): a stride-2 weight AP keeps
            # walrus off the 4-XBUS FWL load path, which otherwise
            # steals stream bandwidth from the moving operand.
            wb = None
            for m in range(2 * NMC):
                mcw, j = divmod(m, 2)
                if j == 0:
                    if mcw < 3:
                        wb = [wb_pre[(mcw, i)] for i in range(KT // 2)]
                    else:
                        wb = []
                        for i in range(KT // 2):
                            eng = nc.sync if i % 2 == 0 else nc.scalar
                            wbk = wpool.tile([128, 4 * CHUNK], bf16,
                                             name="wbk", tag="w")
                            eng.dma_start(wbk[:], wgu[mcw, i])
                            wb.append(wbk)

                pg = ps1.tile([128, B0], f32, name="pg", tag="pg", bufs=2)
                pu = ps1.tile([128, B0], f32, name="pu", tag="pu", bufs=2)
                for k in range(KT):
                    i, half = divmod(k, 2)
                    off = half * 2 * CHUNK
                    st = dict(start=(k == 0), stop=(k == KT - 1))
                    rhs = xt[i][:, half * B0:(half + 1) * B0]
                    nc.tensor.matmul(
                        pg[:], wb[i][:, off + j:off + j + CHUNK - 1:2],
                        rhs, **st
                    )
                    nc.tensor.matmul(
                        pu[:],
                        wb[i][:, off + CHUNK + j:off + CHUNK + j + CHUNK - 1:2],
                        rhs, **st
                    )

                sg = sp.tile([128, B0], f32, name="sg", tag="sg")
                nc.scalar.activation(sg[:], pg[:], SILU)
                g_, idx = divmod(m, 8)
                # aT is written token-interleaved (quarter q of the 512
                # tokens lands at columns (q//2)*256 + (q%2) :: 2) so
                # phase 2's stationary reads are also stride-2.
                for q in range(4):
                    dst = aT[g_][:, idx, (q // 2) * 256 + (q % 2):
                                 (q // 2) * 256 + (q % 2) + 255:2]
                    nc.vector.tensor_mul(
                        dst,
                        sg[:, q * 128:(q + 1) * 128],
                        pu[:, q * 128:(q + 1) * 128],
                    )

            # ---- phase 2: down proj, routing-weight scale in the drain ----
            for nh in range(NH):
                wdk = []
                for i in range(KI // 2):
                    wdkt = wpool.tile([128, 1024], bf16, name="wdk", tag="w")
                    eng = nc.sync if i % 2 == 0 else nc.scalar
                    eng.dma_start(wdkt[:], wd[nh, i])
                    wdk.append(wdkt)
                for mth in range(2):
                    pd = [ps2.tile([128, 512], f32, name="pd", tag="pd", bufs=4)
                          for _ in range(2)]
                    for k in range(KI):
                        i, half = divmod(k, 2)
                        g_, idx = divmod(k, 8)
                        st = dict(start=(k == 0), stop=(k == KI - 1))
                        mv = wdk[i][:, half * 512:(half + 1) * 512]
                        for t2 in range(2):
                            mt = 2 * mth + t2
                            base = (mt // 2) * 256 + (mt % 2)
                            nc.tensor.matmul(
                                pd[t2][:], aT[g_][:, idx, base:base + 255:2],
                                mv, **st
                            )
                    last = (nh == NH - 1 and mth == 1)
                    for t2 in range(2):
                        mt = 2 * mth + t2
                        ot = op.tile([128, 512], f32, name="ot", tag="ot")
                        if t2 == 0:
                            nc.vector.tensor_scalar_mul(ot[:], pd[t2][:],
                                                        rwt[:, mt:mt + 1])
                            eng_o = nc.sync if last else nc.gpsimd
                        else:
                            nc.scalar.mul(ot[:], pd[t2][:], rwt[:, mt:mt + 1])
                            eng_o = nc.scalar if last else nc.gpsimd
                        eng_o.dma_start(
                            dout[mt][:, nh * 512:(nh + 1) * 512], ot[:]
                        )

    nc.compile()
    return nc


@functools.lru_cache(maxsize=2)
def _get_exec(nb1: int):
    """Compile the Bass program and return (nc, run_fn) with a cached jit.

    run_fn(in_maps) -> list of per-core {"d": np.ndarray}. Mirrors
    bass2jax.run_bass_via_pjrt's multi-core branch, but keeps the jitted
    function alive across kernel() calls so repeat invocations skip XLA
    and NEFF compilation.
    """
    import jax
    import concourse.mybir as mybir
    from concourse import bass2jax

    nc = _build(nb1)
    bass2jax.install_neuronx_cc_hook()

    partition_name = nc.partition_id_tensor.name if nc.partition_id_tensor else None
    in_names, out_names, out_avals = [], [], []
    zero_out_shapes = []
    for alloc in nc.m.functions[0].allocations:
        if not isinstance(alloc, mybir.MemoryLocationSet):
            continue
        name = alloc.memorylocations[0].name
        if alloc.kind == "ExternalInput":
            if name != partition_name:
                in_names.append(name)
        elif alloc.kind == "ExternalOutput":
            shape = tuple(alloc.tensor_shape)
            dtype = mybir.dt.np(alloc.dtype)
            out_names.append(name)
            out_avals.append(jax.core.ShapedArray(shape, dtype))
            zero_out_shapes.append((shape, dtype))
    n_params = len(in_names)
    n_outs = len(out_names)
    all_names = list(in_names) + list(out_names)
    if partition_name is not None:
        all_names.append(partition_name)
    donate = tuple(range(n_params, n_params + n_outs))

    def _body(*args):
        operands = list(args)
        if partition_name is not None:
            operands.append(bass2jax.partition_id_tensor())
        outs = bass2jax._bass_exec_p.bind(
            *operands,
            out_avals=tuple(out_avals),
            in_names=tuple(all_names),
            out_names=tuple(out_names),
            lowering_input_output_aliases=(),
            sim_require_finite=True,
            sim_require_nnan=True,
            nc=nc,
        )
        return tuple(outs)

    devices = jax.devices()[:NCORES]
    assert len(devices) == NCORES, f"need {NCORES} devices, have {len(jax.devices())}"
    mesh = bass2jax.Mesh(np.asarray(devices), ("core",))
    in_specs = (bass2jax.PartitionSpec("core"),) * (n_params + n_outs)
    out_specs = (bass2jax.PartitionSpec("core"),) * n_outs
    sharded = jax.jit(
        bass2jax.shard_map(
            _body, mesh=mesh, in_specs=in_specs, out_specs=out_specs, check_rep=False
        ),
        donate_argnums=donate,
        keep_unused=True,
    )

    def run_fn(in_maps):
        concat_in = [
            np.concatenate([np.asarray(m[name]) for m in in_maps], axis=0)
            for name in in_names
        ]
        zeros = [
            np.zeros((shape[0] * NCORES,) + shape[1:], dtype)
            for shape, dtype in zero_out_shapes
        ]
        out_arrs = sharded(*concat_in, *zeros)
        results = []
        for c in range(NCORES):
            res = {}
            for i, name in enumerate(out_names):
                arr = np.asarray(out_arrs[i])
                per = arr.shape[0] // NCORES
                res[name] = arr[c * per:(c + 1) * per]
            results.append(res)
        return results

    return nc, run_fn


def _route(x, w_router):
    """Top-2 routing: expert ids + softmax weights, matching jax.lax.top_k
    (descending, ties to the lower index) + jax.nn.softmax."""
    logits = x.astype(np.float64) @ w_router.astype(np.float64)
    top2 = np.argsort(-logits, axis=1, kind="stable")[:, :TOPK]
    vals = np.take_along_axis(logits, top2, 1).astype(np.float32)
    e = np.exp(vals - vals.max(axis=1, keepdims=True))
    w = (e / e.sum(axis=1, keepdims=True)).astype(np.float32)
    return top2, w


def _reference_numpy(x, w_router, w_gate, w_up, w_down):
    """Correct-but-slow dense fallback for shapes the device program doesn't cover."""
    x = x.astype(np.float32)
    logits = x @ w_router.astype(np.float32)
    n_exp = w_gate.shape[0]
    k = min(TOPK, n_exp)
    top = np.argsort(-logits, axis=1, kind="stable")[:, :k]
    vals = np.take_along_axis(logits, top, 1)
    ex = np.exp(vals - vals.max(1, keepdims=True))
    ww = (ex / ex.sum(1, keepdims=True)).astype(np.float32)
    w_dense = np.zeros_like(logits)
    t_ids = np.arange(x.shape[0])[:, None]
    w_dense[t_ids, top] = ww
    out = np.zeros((x.shape[0], w_down.shape[-1]), np.float32)
    for e in range(n_exp):
        g = x @ w_gate[e]
        u = x @ w_up[e]
        a = (g / (1.0 + np.exp(-g))) * u
        out += w_dense[:, e:e + 1] * (a @ w_down[e])
    return out


def _pack_core_inputs(x, wg_e, wu_e, wd_e, toks, ws, nb1):
    """Build one core's input map: gathered/transposed tokens + packed bf16
    weight tiles matching the SBUF layout, so each DMA tile is one
    contiguous [128 x 2KB-per-partition] run (k-tiles packed in pairs)."""
    assert nb1 == 0
    n_e = len(toks)
    xg = np.zeros((B0, H), np.float32)
    xg[:n_e] = x[toks]
    xTb = np.ascontiguousarray(xg.T).astype(BF16).reshape(KT, 128, B0)
    xTp = np.ascontiguousarray(
        xTb.reshape(KT // 2, 2, 128, B0).transpose(0, 2, 1, 3)
    ).reshape(KT // 2, 128, 2 * B0)
    rfull = np.zeros(B0, np.float32)
    rfull[:n_e] = ws

    def _ileave(block):
        # [..., 256] laid out j0(128)|j1(128) -> column-interleaved
        # j0[c] at 2c, j1[c] at 2c+1 (stride-2 stationary APs defeat FWL)
        lead = block.shape[:-1]
        return np.ascontiguousarray(
            block.reshape(*lead, 2, 128).swapaxes(-2, -1)
        ).reshape(*lead, 2 * 128)

    wgb = wg_e.astype(BF16).reshape(KT, 128, NMC, CHUNK)
    wub = wu_e.astype(BF16).reshape(KT, 128, NMC, CHUNK)
    wgu = np.empty((NMC, KT, 128, 2 * CHUNK), BF16)
    wgu[..., :CHUNK] = _ileave(wgb.transpose(2, 0, 1, 3))
    wgu[..., CHUNK:] = _ileave(wub.transpose(2, 0, 1, 3))
    wgup = np.ascontiguousarray(
        wgu.reshape(NMC, KT // 2, 2, 128, 2 * CHUNK).transpose(0, 1, 3, 2, 4)
    ).reshape(NMC, KT // 2, 128, 4 * CHUNK)

    wdb = wd_e.astype(BF16).reshape(KI, 128, NH, 512).transpose(2, 0, 1, 3)
    wdp = np.ascontiguousarray(
        wdb.reshape(NH, KI // 2, 2, 128, 512).transpose(0, 1, 3, 2, 4)
    ).reshape(NH, KI // 2, 128, 1024)

    return {
        "xT0": xTp,
        "wgu": wgup,
        "wd": wdp,
        "rw": np.ascontiguousarray(rfull.reshape(NT, 128).T),
    }


def kernel(x, w_router, w_gate, w_up, w_down):
    x = np.ascontiguousarray(np.asarray(x, dtype=np.float32))
    w_router = np.asarray(w_router, dtype=np.float32)
    w_gate = np.ascontiguousarray(np.asarray(w_gate, dtype=np.float32))
    w_up = np.ascontiguousarray(np.asarray(w_up, dtype=np.float32))
    w_down = np.ascontiguousarray(np.asarray(w_down, dtype=np.float32))

    if (x.shape != (T, H) or w_router.shape != (H, E)
            or w_gate.shape != (E, H, II) or w_up.shape != (E, H, II)
            or w_down.shape != (E, II, H)):
        return _reference_numpy(x, w_router, w_gate, w_up, w_down)

    top2, w = _route(x, w_router)
    tok = np.repeat(np.arange(T), TOPK)
    te = top2.ravel()
    tw = w.ravel()
    toks_e, ws_e = [], []
    for e in range(E):
        sel = te == e
        toks_e.append(tok[sel])
        ws_e.append(tw[sel].astype(np.float32))

    # Capacity-factor dispatch: the device program handles up to B0=512
    # tokens per expert (98.5% of routed tokens for balanced routing); the
    # rare spill beyond capacity goes through an exact fp32 host path.
    nc, run_fn = _get_exec(0)

    in_maps = [
        _pack_core_inputs(x, w_gate[e], w_up[e], w_down[e],
                          toks_e[e][:B0], ws_e[e][:B0], 0)
        for e in range(E)
    ]

    try:
        results = run_fn(in_maps)
    except Exception:
        import time as _time
        _time.sleep(20)
        results = run_fn(in_maps)

    out = np.zeros((T, H), np.float32)
    for e in range(E):
        n_e = min(len(toks_e[e]), B0)
        d = results[e]["d"].reshape(B0, H)
        out[toks_e[e][:B0]] += d[:n_e]
        spill = toks_e[e][B0:]
        if spill.size:
            xe = x[spill]
            g = xe @ w_gate[e]
            u = xe @ w_up[e]
            a = (g / (1.0 + np.exp(-g))) * u
            out[spill] += (a @ w_down[e]) * ws_e[e][B0:, None]
    return out
